# revision 1
# baseline (speedup 1.0000x reference)
import os
import sys

for _p in ("/opt/trn_rl_repo", "/root/.axon_site/_ro/trn_rl_repo"):
    if os.path.isdir(_p) and _p not in sys.path:
        sys.path.insert(0, _p)

import numpy as np

import concourse.bass as bass
import concourse.mybir as mybir
from concourse.tile import TileContext
from concourse import bass_utils
from concourse import bacc

F32 = mybir.dt.float32
F32R = mybir.dt.float32r
I32 = mybir.dt.int32
AF = mybir.ActivationFunctionType
OP = mybir.AluOpType

N_CORES = 8
BATCH = 65536
C = 4              # classes
T = 120            # time steps
PB = BATCH // N_CORES      # batch per core = 8192
G = 32             # partition groups per class (4*32 = 128 partitions)
FB = PB // G       # free-dim batch per partition = 256
CH = 4             # timesteps per DMA chunk
NS = 1             # independent streams (free-dim split) to hide latency
SW = FB // NS      # stream width
DT_MS = 10.0
EPS = 1e-9
# accumulator is stored scaled: S = 5*acc, so acc_new = max(acc+0.2*(sp-acc),0)
# becomes S_new = max(0.8*S + sp, 0) and the 0.5 threshold becomes 2.5.


def _softplus(x):
    return np.logaddexp(0.0, x.astype(np.float64)).astype(np.float32)


def _build(nc, w00, pb0, inh, ns, input_scale):
    noise_d = nc.dram_tensor("noise", [T // CH, 128, CH * FB], F32, kind="ExternalInput")
    logits_d = nc.dram_tensor("logits_t", [128, FB], F32, kind="ExternalInput")
    w_d = nc.dram_tensor("wmat", [128, 128], F32, kind="ExternalInput")
    out_d = nc.dram_tensor("out", [128, FB], F32, kind="ExternalOutput")

    with TileContext(nc) as tc:
        with (
            tc.tile_pool(name="persist", bufs=1) as persist,
            tc.tile_pool(name="noise", bufs=3) as npool,
            tc.tile_pool(name="work", bufs=3) as work,
            tc.tile_pool(name="psum", bufs=6, space="PSUM") as psum,
        ):
            Wt0 = persist.tile([128, 128], F32)
            nc.sync.dma_start(Wt0[:], w_d[:])
            Wt = persist.tile([128, 128], F32)
            nc.vector.tensor_copy(Wt[:], Wt0[:])
            ev = persist.tile([128, FB], F32)
            lg = persist.tile([128, FB], F32)
            nc.sync.dma_start(lg[:], logits_d[:])
            # evidence = relu(logits*input_scale)*w00 + pb0
            nc.scalar.activation(ev[:], lg[:], AF.Relu, scale=float(input_scale))
            nc.vector.tensor_scalar(ev[:], ev[:], float(w00), float(pb0), OP.mult, OP.add)

            Scur = [persist.tile([128, SW], F32, name=f"Scur{i}") for i in range(NS)]
            Snxt = [persist.tile([128, SW], F32, name=f"Snxt{i}") for i in range(NS)]
            nf = [persist.tile([128, SW], I32, name=f"nf{i}") for i in range(NS)]
            cnt = [persist.tile([128, SW], I32, name=f"cnt{i}") for i in range(NS)]
            Sp = [persist.tile([128, SW], F32, name=f"Sp{i}") for i in range(NS)]
            Sn = [persist.tile([128, SW], F32, name=f"Sn{i}") for i in range(NS)]
            for tls in (Scur, Snxt, cnt, Sp, Sn):
                for tl in tls:
                    nc.vector.memset(tl[:], 0.0)
            for tl in nf:
                nc.vector.memset(tl[:], 1)

            def bookkeeping(s, scur, u, par=0):
                # First-crossing capture: while nf (not-found) is 1, Sp/Sn
                # shadow the pre/post state; nf drops to 0 at the first
                # crossing, freezing them. cnt = sum of nf = crossing index.
                nc.vector.copy_predicated(Sp[s][:], nf[s][:], scur[:])
                nc.vector.copy_predicated(Sn[s][:], nf[s][:], u[:])
                nc.vector.scalar_tensor_tensor(nf[s][:], u[:], 2.5, nf[s][:], OP.is_lt, OP.mult)
                nc.gpsimd.tensor_add(cnt[s][:], cnt[s][:], nf[s][:])

            pend = [None] * NS
            spv = [None] * NS
            Yt = [persist.tile([128, SW], F32, name=f"Ya{i}") for i in range(NS)]
            Yn = [persist.tile([128, SW], F32, name=f"Yb{i}") for i in range(NS)]
            for s in range(NS):
                nc.vector.memset(Yt[s][:], 0.0)   # Ytilde_0 = 0
            for ci in range(T // CH):
                ntile = npool.tile([128, CH * FB], F32)
                nc.sync.dma_start(ntile[:], noise_d[ci])
                for ti in range(CH):
                    for s in range(NS):
                        t = ci * CH + ti
                        cur, nxt = Scur[s], Snxt[s]
                        nslice = ntile[:, ti * FB + s * SW: ti * FB + (s + 1) * SW]
                        evs = ev[:, s * SW:(s + 1) * SW]
                        # off-cycle precombine on Pool: pn = ns*noise + ev,
                        # pn2 = 0.8*Ytilde + pn
                        # noise comes ns-prescaled from the host reshard pass
                        pn = work.tile([128, SW], F32, tag=f"pn{s}", name=f"pn{s}")
                        nc.gpsimd.tensor_add(pn[:], nslice, evs)
                        drive = work.tile([128, SW], F32, tag=f"dr{s}", name=f"dr{s}")
                        if t > 0:
                            # z = W^T sp_{t-1} feeds both drive and Ytilde
                            z = psum.tile([128, SW], F32, tag=f"z{s}", name=f"z{s}")
                            nc.tensor.matmul(z[:], Wt[:], spv[s][:], start=True, stop=True)
                            pn2 = work.tile([128, SW], F32, tag=f"p2{s}", name=f"p2{s}")
                            nc.gpsimd.tensor_add(pn2[:], Yt[s][:], pn[:])
                            # W carries a 0.8 factor, so z' = 0.8*z: undo with 1.25
                            nc.vector.scalar_tensor_tensor(drive[:], z[:], 1.25, pn2[:], OP.mult, OP.add)
                        else:
                            nc.vector.tensor_copy(drive[:], pn[:])
                        if pend[s] is not None:
                            bookkeeping(s, *pend[s])
                        ex = work.tile([128, SW], F32, tag=f"ex{s}", name=f"ex{s}")
                        nc.scalar.activation(ex[:], drive[:], AF.Exp)
                        sp = work.tile([128, SW], F32, tag=f"sp{s}", name=f"sp{s}", bufs=3)
                        nc.scalar.activation(sp[:], ex[:], AF.Ln, bias=1.0)
                        spv[s] = sp
                        if t > 0:
                            # Ytilde_t = 0.8*Ytilde_{t-1} + z (off-cycle; feeds pn2_{t+1})
                            nc.vector.scalar_tensor_tensor(Yn[s][:], Yt[s][:], 0.8, z[:], OP.mult, OP.add)
                            Yt[s], Yn[s] = Yn[s], Yt[s]
                        # u = 0.8*S + sp IS the new state (never negative, the
                        # reference's max(.,0) is dead code) - off the cycle.
                        nc.vector.scalar_tensor_tensor(nxt[:], cur[:], 0.8, sp[:], OP.mult, OP.add)
                        pend[s] = (cur, nxt, t % 2)
                        Scur[s], Snxt[s] = nxt, cur
            for s in range(NS):
                bookkeeping(s, *pend[s])
            for s in range(NS):
                # idx = cnt (sum of not-found flags); idx0 = max(idx-1, 0)
                fnd = work.tile([128, SW], F32, tag=f"fd{s}")
                nc.vector.tensor_scalar(fnd[:], nf[s][:], -1.0, 1.0, OP.mult, OP.add)
                idx = work.tile([128, SW], F32, tag=f"t1{s}")
                nc.vector.tensor_scalar(idx[:], cnt[s][:], 1.0, None, OP.mult)
                idx0 = work.tile([128, SW], F32, tag=f"dr{s}")
                nc.vector.tensor_scalar(idx0[:], idx[:], 1.0, 0.0, OP.subtract, OP.max)
                # frac = (2.5 - Sp) / (Sn - Sp + 5*EPS), zeroed when idx == 0
                den = work.tile([128, SW], F32, tag=f"sp{s}")
                nc.vector.tensor_sub(den[:], Sn[s][:], Sp[s][:])
                nc.vector.tensor_scalar(den[:], den[:], 5.0 * EPS, None, OP.add)
                rec = work.tile([128, SW], F32, tag=f"u{s}")
                nc.vector.reciprocal(rec[:], den[:])
                num = work.tile([128, SW], F32, tag=f"nm{s}")
                nc.vector.tensor_scalar(num[:], Sp[s][:], -1.0, 2.5, OP.mult, OP.add)
                frac = work.tile([128, SW], F32, tag=f"fr{s}")
                nc.vector.tensor_mul(frac[:], num[:], rec[:])
                mi = work.tile([128, SW], F32, tag=f"mi{s}")
                nc.vector.tensor_scalar(mi[:], idx[:], 0.5, None, OP.is_ge)
                nc.vector.tensor_mul(frac[:], frac[:], mi[:])
                tval = work.tile([128, SW], F32, tag=f"tv{s}")
                nc.vector.tensor_add(tval[:], idx0[:], frac[:])
                # out_sec = found ? tval*DT/1000 : T*DT/1000
                tmax = T * DT_MS / 1000.0
                nc.vector.tensor_scalar(tval[:], tval[:], DT_MS / 1000.0, -tmax, OP.mult, OP.add)
                nc.vector.tensor_mul(tval[:], tval[:], fnd[:])
                nc.vector.tensor_scalar(tval[:], tval[:], tmax, None, OP.add)
                nc.sync.dma_start(out_d[:, s * SW:(s + 1) * SW], tval[:])
    return nc


def _pin_act_table(nc):
    # All activation funcs used (Exp, Ln, Relu, Copy) live together in the
    # natural_log_exp_and_others set; blank the others (keeping list indices,
    # which are the runtime set ids) so the chooser can't ping-pong tables
    # inside the scan loop.
    from concourse import hw_specs as _hs
    import concourse.bacc as _bacc
    full = dict(_hs.get_activation_tables(nc.m.arch))
    keep = "natural_log_exp_and_others"
    patched = {k: (v if k == keep else set()) for k, v in full.items()}
    _bacc.get_activation_tables = lambda arch: patched


last_results = None


def kernel(logits, input_scale, leak, self_excitation, inhibition, noise_std,
           proj_w, proj_b, noise_base):
    logits = np.asarray(logits, dtype=np.float32)
    noise_base = np.asarray(noise_base, dtype=np.float32)
    lk = _softplus(np.asarray(leak))
    se = _softplus(np.asarray(self_excitation))
    inh = float(_softplus(np.asarray(inhibition)))
    ns = float(_softplus(np.asarray(noise_std)))
    alpha = se + inh - lk  # [C]
    w00 = float(np.asarray(proj_w)[0, 0])
    pb0 = float(np.asarray(proj_b)[0])
    iscale = float(np.asarray(input_scale))

    # W[p,q] = (alpha[class(q)]*(p==q) - inh*(p%G==q%G)) / 5
    p_idx = np.arange(128)
    q_idx = np.arange(128)
    Wm = (-inh / 5.0) * (p_idx[:, None] % G == q_idx[None, :] % G).astype(np.float32)
    Wm[q_idx, q_idx] += alpha[q_idx // G] / 5.0
    Wm *= 0.8  # Ys-recurrence scaling: z' = 0.8*z

    nc = bacc.Bacc("TRN2", target_bir_lowering=False, debug=False, num_devices=N_CORES)
    _build(nc, w00, pb0, inh, ns, iscale)
    _pin_act_table(nc)
    nc.compile()

    in_maps = []
    for c in range(N_CORES):
        s = c * PB
        nz = noise_base[:, s:s + PB, :].reshape(T, G, FB, C) * np.float32(ns)
        nz = np.ascontiguousarray(nz.transpose(0, 3, 1, 2)).reshape(T, 128, FB)
        nz = np.ascontiguousarray(
            nz.reshape(T // CH, CH, 128, FB).transpose(0, 2, 1, 3)
        ).reshape(T // CH, 128, CH * FB)
        lg = logits[s:s + PB].reshape(G, FB, C)
        lg = np.ascontiguousarray(lg.transpose(2, 0, 1)).reshape(128, FB)
        in_maps.append({"noise": nz, "logits_t": lg, "wmat": Wm})

    res = bass_utils.run_bass_kernel_spmd(nc, in_maps, core_ids=list(range(N_CORES)))
    global last_results
    last_results = res
    outs = []
    for c in range(N_CORES):
        o = res.results[c]["out"].reshape(C, G, FB)
        outs.append(o.transpose(1, 2, 0).reshape(PB, C))
    return np.concatenate(outs, axis=0)



# revision 13
# speedup vs baseline: 1.5033x; 1.5033x over previous
import os
import sys

for _p in ("/opt/trn_rl_repo", "/root/.axon_site/_ro/trn_rl_repo"):
    if os.path.isdir(_p) and _p not in sys.path:
        sys.path.insert(0, _p)

import numpy as np

import concourse.bass as bass
import concourse.mybir as mybir
from concourse.tile import TileContext
from concourse import bass_utils
from concourse import bacc

F16 = mybir.dt.float16
F32 = mybir.dt.float32
I16 = mybir.dt.int16
BF16 = mybir.dt.bfloat16
I32D = mybir.dt.int32
AF = mybir.ActivationFunctionType
OP = mybir.AluOpType

N_CORES = 8
BATCH = 65536
C = 4                  # classes
T = 120                # time steps
PB = BATCH // N_CORES  # batch per core = 8192
G = 32                 # groups (partitions = 4 classes x 32 groups)
FB = PB // G           # free-dim batch per partition = 256
K = 16                 # epoch length (0.8^-u stays fp16-safe)
NE = T // K            # epochs minus... T=120 -> 7 full epochs + 8 steps
OCT = 8                # octet length for bookkeeping
NOCT = T // OCT        # 15 octets
CH = 8                 # dpn steps per DMA chunk
DT_MS = 10.0
THR = 2.5              # threshold in S=5*acc units
EPS5 = 5e-9
FILL = 448             # filler matmul columns (PE p-state keep-alive)
DEC = 0.8


def _softplus(x):
    return np.logaddexp(0.0, x.astype(np.float64)).astype(np.float32)


def _build(nc):
    dpn_d = nc.dram_tensor("dpn", [T // CH, 128, CH * FB], F16, kind="ExternalInput")
    w_d = nc.dram_tensor("wmats", [128, 17 * 128], F16, kind="ExternalInput")
    out_d = nc.dram_tensor("out", [128, FB], F32, kind="ExternalOutput")

    with TileContext(nc) as tc:
        with (
            tc.tile_pool(name="persist", bufs=1) as persist,
            tc.tile_pool(name="dpnp", bufs=3) as dpnp,
            tc.tile_pool(name="work", bufs=2) as work,
            tc.tile_pool(name="psum", bufs=1, space="PSUM") as psump,
        ):
            # --- stationary weights: W_j (j=0..14), W_m1=A, Wcar=0.8*A ---
            wall = persist.tile([128, 17 * 128], F16, name="wall")
            nc.sync.dma_start(wall[:], w_d[:])
            Wj = [wall[:, j * 128:(j + 1) * 128] for j in range(15)]
            Wm1 = wall[:, 15 * 128:16 * 128]
            Wcar = wall[:, 16 * 128:17 * 128]
            I32 = mybir.dt.int32
            ident = persist.tile([128, 128], F16, name="ident")
            rowi = persist.tile([128, 128], I32, name="rowi")
            nc.gpsimd.iota(rowi[:], [[1, 128]], base=0, channel_multiplier=0)
            coli = persist.tile([128, 1], I32, name="coli")
            nc.gpsimd.iota(coli[:], [[0, 1]], base=0, channel_multiplier=1)
            row = persist.tile([128, 128], F32, name="row")
            nc.vector.tensor_copy(row[:], rowi[:])
            col = persist.tile([128, 1], F32, name="col")
            nc.vector.tensor_copy(col[:], coli[:])
            nc.vector.tensor_scalar(ident[:], row[:], col[:], None, OP.is_equal)

            # --- state rings ---
            spr = persist.tile([128, 16 * FB], F16, name="spr")   # sp ring, 16 slots
            Sr = persist.tile([128, 16 * FB], F16, name="Sr")     # S ring, 16 slots
            nc.vector.memset(spr[:], 0.0)
            nc.vector.memset(Sr[:], 0.0)
            QM = persist.tile([128, FB], F16, name="QM")          # max of [S>=thr]*(120-t)
            nc.vector.memset(QM[:], 0.0)
            cnt = persist.tile([128, FB], I32D, name="cnt")        # octets before crossing
            nc.vector.memset(cnt[:], 0)
            capS = persist.tile([128, FB], F16, name="capS")      # S before crossing octet
            capSP = persist.tile([128, OCT * FB], F16, name="capSP")  # sp's of crossing octet
            nc.vector.memset(capS[:], 0.0)
            nc.vector.memset(capSP[:], 0.0)
            nfo = persist.tile([128, FB], I32D, name="nfo")        # not-crossed @ octet start
            nfow = persist.tile([128, OCT * FB], I16, name="nfow")  # wide copy of nfo
            fseed = persist.tile([128, FILL], F16, name="fseed")
            nc.vector.memset(fseed[:], 0.125)

            # --- PSUM accumulators ---
            P0 = psump.tile([128, FB], F32, name="P0")
            P1 = psump.tile([128, FB], F32, name="P1")
            E0 = psump.tile([128, FB], F32, name="E0")
            E1 = psump.tile([128, FB], F32, name="E1")
            FP = psump.tile([128, FILL], F32, name="FP")
            Ps = [P0, P1]

            def fill_mm(n=1):
                for _ in range(n):
                    nc.tensor.matmul(FP[:], ident[:], fseed[:], start=True, stop=True)

            # warm up PE so it ramps to full speed before the scan
            for _ in range(10):
                fill_mm()

            dpn_t = [None] * (T // CH)
            spv = None  # sp tile AP of previous step (ring slot view)

            for t in range(T):
                u = t % K
                e = t // K
                P = Ps[e % 2]
                Pn = Ps[(e + 1) % 2]
                Etile = E0 if (t % 2 == 0) else E1

                if t % OCT == 0:
                    # octet start: latch not-crossed flag BEFORE this step's
                    # QM update; accumulate octet count; widen for captures
                    nc.vector.tensor_scalar(nfo[:], QM[:], 0.5, None, OP.is_lt)
                    nc.gpsimd.tensor_tensor(cnt[:], cnt[:], nfo[:], OP.add)
                    nc.vector.tensor_copy(
                        nfow[:].rearrange("p (a b) -> p a b", a=OCT),
                        nfo[:].unsqueeze(1).broadcast_to([128, OCT, FB]),
                    )

                ci = t // CH
                if t % CH == 0:
                    dtile = dpnp.tile([128, CH * FB], F16, tag="dpn", name=f"dp{ci}")
                    nc.sync.dma_start(dtile[:], dpn_d[ci])
                    dpn_t[ci] = dtile
                dslice = dpn_t[ci][:, (t % CH) * FB:(t % CH + 1) * FB]

                # --- PE: drive accumulation in PSUM ---
                if t == 0:
                    nc.tensor.matmul(P[:], ident[:], dslice, start=True, stop=False)
                else:
                    if u == 0:
                        # first z of the epoch went to this P already (W_m1);
                        # dpn joins the running accumulation
                        nc.tensor.matmul(P[:], ident[:], dslice, start=False, stop=False)
                    else:
                        nc.tensor.matmul(P[:], Wj[u - 1], spv, start=False, stop=False)
                        nc.tensor.matmul(P[:], ident[:], dslice, start=False, stop=False)

                # --- Act: softplus = Ln(1 + Exp(scale * P)) ---
                sslot = t % 16
                spc = spr[:, sslot * FB:(sslot + 1) * FB]
                nc.scalar.activation(Etile[:], P[:], AF.Exp, scale=float(DEC ** u))
                nc.scalar.activation(spc, Etile[:], AF.Ln, bias=1.0)
                spv = spc

                # --- PE: epoch handoff (emitted after Act so program order is ok) ---
                if u == K - 1 and t < T - 1:
                    # next epoch's P: start with carry 0.8*A^T S_{t-1-0}?? uses S_{t-1}
                    # carry = 0.8*A^T S_{t-1}  (S of step t-1 = slot (t-1)%16)
                    sprev = Sr[:, ((t - 1) % 16) * FB:((t - 1) % 16 + 1) * FB]
                    nc.tensor.matmul(Pn[:], Wcar, sprev, start=True, stop=False)
                    # z of this step's sp goes to next epoch with W_m1 = A
                    nc.tensor.matmul(Pn[:], Wm1, spc, start=False, stop=False)

                # --- DVE: S update (writes S ring) ---
                Sprev = Sr[:, ((t - 1) % 16) * FB:((t - 1) % 16 + 1) * FB]
                Scur = Sr[:, (t % 16) * FB:((t % 16) + 1) * FB]
                if t == 0:
                    nc.vector.tensor_scalar(Scur, spc, 1.0, None, OP.mult)
                else:
                    nc.vector.scalar_tensor_tensor(Scur, Sprev, DEC, spc, OP.mult, OP.add)

                # --- DVE: q = [S >= thr] * (T - t);  Pool: QM = max(QM, q) ---
                q = work.tile([128, FB], F16, tag="q", name=f"q{t}")
                nc.vector.tensor_scalar(q[:], Scur, THR, float(T - t), OP.is_ge, OP.mult)
                nc.vector.tensor_tensor(QM[:], QM[:], q[:], OP.max)

                # --- octet-end capture ---
                if t % OCT == OCT - 1:
                    o = t // OCT
                    half = (o % 2) * OCT
                    nc.vector.copy_predicated(
                        capSP[:], nfow[:], spr[:, half * FB:(half + OCT) * FB])
                    spre = Sr[:, ((8 * o - 1) % 16) * FB:(((8 * o - 1) % 16) + 1) * FB]
                    nc.vector.copy_predicated(capS[:], nfo[:], spre)

                # --- PE filler to keep the p-state hot ---
                fill_mm(1)

            # ================= epilogue =================
            # reconstruct R_j (S values inside the crossing octet)
            Rj = []
            Rprev = capS
            for j in range(OCT):
                R = persist.tile([128, FB], F16, name=f"R{j}")
                nc.vector.scalar_tensor_tensor(
                    R[:], Rprev[:] if j == 0 else Rprev, DEC,
                    capSP[:, j * FB:(j + 1) * FB], OP.mult, OP.add)
                Rj.append(R)
                Rprev = R[:]

            idx = work.tile([128, FB], F32, tag="ep", name="idx")
            nc.vector.tensor_scalar(idx[:], QM[:], -1.0, float(T), OP.mult, OP.add)
            cntf = work.tile([128, FB], F32, tag="ep4", name="cntf")
            nc.vector.tensor_copy(cntf[:], cnt[:])
            jstar = work.tile([128, FB], F32, tag="ep2", name="jstar")
            # j* = idx - 8*(cnt-1) = idx + (-8*cnt + 8)
            tmp8 = work.tile([128, FB], F32, tag="ep3", name="tmp8")
            nc.vector.tensor_scalar(tmp8[:], cntf[:], -8.0, 8.0, OP.mult, OP.add)
            nc.vector.tensor_tensor(jstar[:], idx[:], tmp8[:], OP.add)

            Sn = persist.tile([128, FB], F16, name="Sn")
            Sp = persist.tile([128, FB], F16, name="Sp")
            nc.vector.memset(Sn[:], 1.0)
            nc.vector.memset(Sp[:], 0.0)
            for j in range(OCT):
                pj = work.tile([128, FB], I16, tag="pj", name=f"pj{j}")
                nc.vector.tensor_scalar(pj[:], jstar[:], float(j), None, OP.is_equal)
                nc.vector.copy_predicated(Sn[:], pj[:], Rj[j][:])
                nc.vector.copy_predicated(Sp[:], pj[:], capS[:] if j == 0 else Rj[j - 1][:])

            # frac and final time (fp32)
            Sp32 = work.tile([128, FB], F32, tag="f1", name="Sp32")
            nc.vector.tensor_copy(Sp32[:], Sp[:])
            Sn32 = work.tile([128, FB], F32, tag="f2", name="Sn32")
            nc.vector.tensor_copy(Sn32[:], Sn[:])
            den = work.tile([128, FB], F32, tag="f3", name="den")
            nc.vector.tensor_tensor(den[:], Sn32[:], Sp32[:], OP.subtract)
            nc.vector.tensor_scalar(den[:], den[:], EPS5, None, OP.add)
            rec = work.tile([128, FB], F32, tag="f4", name="rec")
            nc.vector.reciprocal(rec[:], den[:])
            num = work.tile([128, FB], F32, tag="f5", name="num")
            nc.vector.tensor_scalar(num[:], Sp32[:], -1.0, THR, OP.mult, OP.add)
            frac = work.tile([128, FB], F32, tag="f6", name="frac")
            nc.vector.tensor_tensor(frac[:], num[:], rec[:], OP.mult)
            # zero frac when idx == 0
            mi = work.tile([128, FB], F32, tag="f7", name="mi")
            nc.vector.tensor_scalar(mi[:], idx[:], 0.5, None, OP.is_ge)
            nc.vector.tensor_tensor(frac[:], frac[:], mi[:], OP.mult)
            idx0 = work.tile([128, FB], F32, tag="f8", name="idx0")
            nc.vector.tensor_scalar(idx0[:], idx[:], 1.0, 0.0, OP.subtract, OP.max)
            tval = work.tile([128, FB], F32, tag="f9", name="tval")
            nc.vector.tensor_tensor(tval[:], idx0[:], frac[:], OP.add)
            # seconds = found * (tval*0.01 - 1.2) + 1.2
            nc.vector.tensor_scalar(tval[:], tval[:], DT_MS / 1000.0, -1.2, OP.mult, OP.add)
            fnd = work.tile([128, FB], F32, tag="fa", name="fnd")
            nc.vector.tensor_scalar(fnd[:], QM[:], 0.5, None, OP.is_ge)
            nc.vector.tensor_tensor(tval[:], tval[:], fnd[:], OP.mult)
            nc.vector.tensor_scalar(tval[:], tval[:], 1.2, None, OP.add)
            nc.sync.dma_start(out_d[:], tval[:])
    return nc


def _pin_act_table(nc):
    from concourse import hw_specs as _hs
    import concourse.bacc as _bacc
    full = dict(_hs.get_activation_tables(nc.m.arch))
    keep = "natural_log_exp_and_others"
    patched = {k: (v if k == keep else set()) for k, v in full.items()}
    _bacc.get_activation_tables = lambda arch: patched


last_results = None


def kernel(logits, input_scale, leak, self_excitation, inhibition, noise_std,
           proj_w, proj_b, noise_base):
    logits = np.asarray(logits, dtype=np.float32)
    noise_base = np.asarray(noise_base, dtype=np.float32)
    lk = _softplus(np.asarray(leak))
    se = _softplus(np.asarray(self_excitation))
    inh = float(_softplus(np.asarray(inhibition)))
    ns = float(_softplus(np.asarray(noise_std)))
    alpha = se + inh - lk  # [C]
    w00 = float(np.asarray(proj_w)[0, 0])
    pb0 = float(np.asarray(proj_b)[0])
    iscale = float(np.asarray(input_scale))

    # evidence + scaled noise, fp32 on host
    ev = (np.maximum(logits * iscale, 0.0) * w00 + pb0).astype(np.float32)  # [B,C]
    pn = noise_base * np.float32(ns) + ev[None, :, :]                       # [T,B,C]

    # A-tilde matrix on the 128-partition layout: partition p = c*G + g
    p_idx = np.arange(128)
    q_idx = np.arange(128)
    Am = (-inh / 5.0) * (p_idx[:, None] % G == q_idx[None, :] % G).astype(np.float32)
    Am[q_idx, q_idx] += alpha[q_idx // G] / 5.0
    wmats3 = np.zeros((17, 128, 128), dtype=np.float16)
    for j in range(15):
        wmats3[j] = (Am * (DEC ** (-(j + 1)))).astype(np.float16)
    wmats3[15] = Am.astype(np.float16)          # W_m1
    wmats3[16] = (Am * DEC).astype(np.float16)  # Wcar
    wmats = np.ascontiguousarray(wmats3.transpose(1, 0, 2)).reshape(128, 17 * 128)

    # dpn: epoch-scaled noise increments, fp16
    # dpn_t = 0.8^-u * pn_t - [u>0] * 0.8^-(u-1) * pn_{t-1},  u = t % K
    u_arr = np.arange(T) % K
    sc = (DEC ** (-u_arr.astype(np.float64))).astype(np.float32)          # [T]
    dpn = pn * sc[:, None, None]
    dpn[1:][u_arr[1:] > 0] -= pn[:-1][u_arr[1:] > 0] * sc[:-1][u_arr[1:] > 0, None, None]
    dpn = dpn.astype(np.float16)

    nc = bacc.Bacc("TRN2", target_bir_lowering=False, debug=False, num_devices=N_CORES)
    _build(nc)
    _pin_act_table(nc)
    nc.compile()

    in_maps = []
    for c in range(N_CORES):
        s = c * PB
        nz = dpn[:, s:s + PB, :].reshape(T, G, FB, C)
        nz = np.ascontiguousarray(nz.transpose(0, 3, 1, 2)).reshape(T, 128, FB)
        nz = np.ascontiguousarray(
            nz.reshape(T // CH, CH, 128, FB).transpose(0, 2, 1, 3)
        ).reshape(T // CH, 128, CH * FB)
        in_maps.append({"dpn": nz, "wmats": wmats})

    res = bass_utils.run_bass_kernel_spmd(nc, in_maps, core_ids=list(range(N_CORES)))
    global last_results
    last_results = res
    outs = []
    for c in range(N_CORES):
        o = res.results[c]["out"].reshape(C, G, FB)
        outs.append(o.transpose(1, 2, 0).reshape(PB, C))
    return np.concatenate(outs, axis=0)


# revision 15
# speedup vs baseline: 1.5780x; 1.0497x over previous
import os
import sys

for _p in ("/opt/trn_rl_repo", "/root/.axon_site/_ro/trn_rl_repo"):
    if os.path.isdir(_p) and _p not in sys.path:
        sys.path.insert(0, _p)

import numpy as np

import concourse.bass as bass
import concourse.mybir as mybir
from concourse.tile import TileContext
from concourse import bass_utils
from concourse import bacc

F16 = mybir.dt.float16
F32 = mybir.dt.float32
I16 = mybir.dt.int16
BF16 = mybir.dt.bfloat16
I32D = mybir.dt.int32
AF = mybir.ActivationFunctionType
OP = mybir.AluOpType

N_CORES = 8
BATCH = 65536
C = 4                  # classes
T = 120                # time steps
PB = BATCH // N_CORES  # batch per core = 8192
G = 32                 # groups (partitions = 4 classes x 32 groups)
FB = PB // G           # free-dim batch per partition = 256
K = 16                 # epoch length (0.8^-u stays fp16-safe)
NE = T // K            # epochs minus... T=120 -> 7 full epochs + 8 steps
OCT = 8                # octet length for bookkeeping
NOCT = T // OCT        # 15 octets
CH = 8                 # dpn steps per DMA chunk
DT_MS = 10.0
THR = 2.5              # threshold in S=5*acc units
EPS5 = 5e-9
FILL = 448             # filler matmul columns (PE p-state keep-alive)
DEC = 0.8


def _softplus(x):
    return np.logaddexp(0.0, x.astype(np.float64)).astype(np.float32)


def _build(nc):
    dpn_d = nc.dram_tensor("dpn", [T // CH, 128, CH * FB], F16, kind="ExternalInput")
    w_d = nc.dram_tensor("wmats", [128, 17 * 128], F16, kind="ExternalInput")
    out_d = nc.dram_tensor("out", [128, FB], F32, kind="ExternalOutput")

    with TileContext(nc) as tc:
        with (
            tc.tile_pool(name="persist", bufs=1) as persist,
            tc.tile_pool(name="dpnp", bufs=3) as dpnp,
            tc.tile_pool(name="work", bufs=2) as work,
            tc.tile_pool(name="psum", bufs=1, space="PSUM") as psump,
        ):
            # --- stationary weights: W_j (j=0..14), W_m1=A, Wcar=0.8*A ---
            wall = persist.tile([128, 17 * 128], F16, name="wall")
            nc.sync.dma_start(wall[:], w_d[:])
            Wj = [wall[:, j * 128:(j + 1) * 128] for j in range(15)]
            Wm1 = wall[:, 15 * 128:16 * 128]
            Wcar = wall[:, 16 * 128:17 * 128]
            I32 = mybir.dt.int32
            ident = persist.tile([128, 128], F16, name="ident")
            rowi = persist.tile([128, 128], I32, name="rowi")
            nc.gpsimd.iota(rowi[:], [[1, 128]], base=0, channel_multiplier=0)
            coli = persist.tile([128, 1], I32, name="coli")
            nc.gpsimd.iota(coli[:], [[0, 1]], base=0, channel_multiplier=1)
            row = persist.tile([128, 128], F32, name="row")
            nc.vector.tensor_copy(row[:], rowi[:])
            col = persist.tile([128, 1], F32, name="col")
            nc.vector.tensor_copy(col[:], coli[:])
            nc.vector.tensor_scalar(ident[:], row[:], col[:], None, OP.is_equal)

            # --- state rings ---
            spr = persist.tile([128, 16 * FB], F16, name="spr")   # sp ring, 16 slots
            Sr = persist.tile([128, 16 * FB], F16, name="Sr")     # S ring, 16 slots
            nc.vector.memset(Sr[:], 0.0)
            Fo = persist.tile([128, FB], F16, name="Fo")          # crossed-by-octet flag
            nc.vector.memset(Fo[:], 0.0)
            cnt = persist.tile([128, FB], I32D, name="cnt")        # octets before crossing
            nc.vector.memset(cnt[:], 0)
            capS = persist.tile([128, FB], F16, name="capS")      # S before crossing octet
            capSP = persist.tile([128, OCT * FB], F16, name="capSP")  # sp's of crossing octet
            nc.vector.memset(capS[:], 0.0)
            nc.vector.memset(capSP[:], 0.0)
            nfo = persist.tile([128, FB], I32D, name="nfo")        # not-crossed @ octet start
            nfow = persist.tile([128, OCT * FB], I16, name="nfow")  # wide copy of nfo
            fseed = persist.tile([128, FILL], F16, name="fseed")
            nc.vector.memset(fseed[:], 0.125)

            # --- PSUM accumulators ---
            P0 = psump.tile([128, FB], F32, name="P0")
            P1 = psump.tile([128, FB], F32, name="P1")
            E0 = psump.tile([128, FB], F32, name="E0")
            E1 = psump.tile([128, FB], F32, name="E1")
            FP = psump.tile([128, FILL], F32, name="FP")
            Ps = [P0, P1]

            def fill_mm(n=1):
                for _ in range(n):
                    nc.tensor.matmul(FP[:], ident[:], fseed[:], start=True, stop=True)

            # warm up PE so it ramps to full speed before the scan
            for _ in range(10):
                fill_mm()

            dpn_t = [None] * (T // CH)
            dt0 = dpnp.tile([128, CH * FB], F16, tag="dpn", name="dp0")
            nc.sync.dma_start(dt0[:], dpn_d[0])
            dpn_t[0] = dt0
            nc.tensor.matmul(P0[:], ident[:], dt0[:, 0:FB], start=True, stop=False)
            spv = None  # sp tile AP of previous step (ring slot view)

            for t in range(T):
                u = t % K
                e = t // K
                P = Ps[e % 2]
                Pn = Ps[(e + 1) % 2]
                Etile = E0 if (t % 2 == 0) else E1

                if t % OCT == 0:
                    # octet start: latch not-crossed flag BEFORE this step's
                    # QM update; accumulate octet count; widen for captures
                    nc.vector.tensor_scalar(nfo[:], Fo[:], 0.5, None, OP.is_lt)
                    nc.gpsimd.tensor_tensor(cnt[:], cnt[:], nfo[:], OP.add)
                    nc.vector.tensor_copy(
                        nfow[:].rearrange("p (a b) -> p a b", a=OCT),
                        nfo[:].unsqueeze(1).broadcast_to([128, OCT, FB]),
                    )

                # --- PE: carry for next epoch (start=True on Pn) must precede
                # the prefetched dpn matmul targeting Pn ---
                if u == K - 1 and t < T - 1:
                    sprev2 = Sr[:, ((t - 1) % 16) * FB:((t - 1) % 16 + 1) * FB]
                    nc.tensor.matmul(Pn[:], Wcar, sprev2, start=True, stop=False)

                # --- PE: z of previous sp into current P ---
                if t > 0 and u > 0:
                    nc.tensor.matmul(P[:], Wj[u - 1], spv, start=False, stop=False)

                # --- Act: softplus = Ln(1 + Exp(scale * P)) ---
                sslot = t % 16
                spc = spr[:, sslot * FB:(sslot + 1) * FB]
                nc.scalar.activation(Etile[:], P[:], AF.Exp, scale=float(DEC ** u))
                nc.scalar.activation(spc, Etile[:], AF.Ln, bias=1.0)
                spv = spc

                # --- PE: prefetch dpn matmul for step t+1 (runs during Ln_t,
                # emitted after Exp_t so the WAR dep points the right way) ---
                if t + 1 < T:
                    cin = (t + 1) // CH
                    if (t + 1) % CH == 0:
                        dtile = dpnp.tile([128, CH * FB], F16, tag="dpn", name=f"dp{cin}")
                        nc.sync.dma_start(dtile[:], dpn_d[cin])
                        dpn_t[cin] = dtile
                    dsl_n = dpn_t[cin][:, ((t + 1) % CH) * FB:((t + 1) % CH + 1) * FB]
                    un = (t + 1) % K
                    Ptgt = Pn if un == 0 else P
                    nc.tensor.matmul(Ptgt[:], ident[:], dsl_n, start=False, stop=False)

                # --- PE: last z of the epoch feeds the next epoch's P ---
                if u == K - 1 and t < T - 1:
                    nc.tensor.matmul(Pn[:], Wm1, spc, start=False, stop=False)

                # --- DVE: S update (writes S ring) ---
                Sprev = Sr[:, ((t - 1) % 16) * FB:((t - 1) % 16 + 1) * FB]
                Scur = Sr[:, (t % 16) * FB:((t % 16) + 1) * FB]
                if t == 0:
                    nc.vector.tensor_scalar(Scur, spc, 1.0, None, OP.mult)
                else:
                    nc.vector.scalar_tensor_tensor(Scur, Sprev, DEC, spc, OP.mult, OP.add)

                # --- octet-end: max tree over this octet's S slots + capture ---
                if t % OCT == OCT - 1:
                    o = t // OCT
                    half = (o % 2) * OCT
                    SrH = Sr[:, half * FB:(half + OCT) * FB]
                    L1 = work.tile([128, 4 * FB], F16, tag="L1", name=f"L1_{o}")
                    nc.vector.tensor_tensor(
                        L1[:], SrH[:, 0:4 * FB], SrH[:, 4 * FB:8 * FB], OP.max)
                    L2 = work.tile([128, 2 * FB], F16, tag="L2", name=f"L2_{o}")
                    nc.vector.tensor_tensor(
                        L2[:], L1[:, 0:2 * FB], L1[:, 2 * FB:4 * FB], OP.max)
                    L3 = work.tile([128, FB], F16, tag="L3", name=f"L3_{o}")
                    nc.vector.tensor_tensor(L3[:], L2[:, 0:FB], L2[:, FB:2 * FB], OP.max)
                    nc.vector.scalar_tensor_tensor(Fo[:], L3[:], THR, Fo[:], OP.is_ge, OP.max)
                    nc.vector.copy_predicated(
                        capSP[:], nfow[:], spr[:, half * FB:(half + OCT) * FB])
                    spre = Sr[:, ((8 * o - 1) % 16) * FB:(((8 * o - 1) % 16) + 1) * FB]
                    nc.vector.copy_predicated(capS[:], nfo[:], spre)

                # --- PE filler to keep the p-state hot ---
                fill_mm(1)

            # ================= epilogue =================
            # reconstruct R_j (S values inside the crossing octet)
            Rj = []
            Rprev = capS
            for j in range(OCT):
                R = persist.tile([128, FB], F16, name=f"R{j}")
                nc.vector.scalar_tensor_tensor(
                    R[:], Rprev[:] if j == 0 else Rprev, DEC,
                    capSP[:, j * FB:(j + 1) * FB], OP.mult, OP.add)
                Rj.append(R)
                Rprev = R[:]

            # j* = count of leading below-threshold cummax over R_j
            jstar = work.tile([128, FB], F32, tag="ep2", name="jstar")
            nc.vector.memset(jstar[:], 0.0)
            cm = work.tile([128, FB], F16, tag="ep5", name="cm")
            for j in range(OCT):
                if j == 0:
                    nc.vector.tensor_copy(cm[:], Rj[0][:])
                else:
                    nc.vector.tensor_tensor(cm[:], cm[:], Rj[j][:], OP.max)
                nc.vector.scalar_tensor_tensor(
                    jstar[:], cm[:], THR, jstar[:], OP.is_lt, OP.add)
            cntf = work.tile([128, FB], F32, tag="ep4", name="cntf")
            nc.vector.tensor_copy(cntf[:], cnt[:])
            # idx = 8*(cnt-1) + j*
            idx = work.tile([128, FB], F32, tag="ep", name="idx")
            nc.vector.tensor_scalar(idx[:], cntf[:], 8.0, -8.0, OP.mult, OP.add)
            nc.vector.tensor_tensor(idx[:], idx[:], jstar[:], OP.add)

            Sn = persist.tile([128, FB], F16, name="Sn")
            Sp = persist.tile([128, FB], F16, name="Sp")
            nc.vector.memset(Sn[:], 1.0)
            nc.vector.memset(Sp[:], 0.0)
            for j in range(OCT):
                pj = work.tile([128, FB], I16, tag="pj", name=f"pj{j}")
                nc.vector.tensor_scalar(pj[:], jstar[:], float(j), None, OP.is_equal)
                nc.vector.copy_predicated(Sn[:], pj[:], Rj[j][:])
                nc.vector.copy_predicated(Sp[:], pj[:], capS[:] if j == 0 else Rj[j - 1][:])

            # frac and final time (fp32)
            Sp32 = work.tile([128, FB], F32, tag="f1", name="Sp32")
            nc.vector.tensor_copy(Sp32[:], Sp[:])
            Sn32 = work.tile([128, FB], F32, tag="f2", name="Sn32")
            nc.vector.tensor_copy(Sn32[:], Sn[:])
            den = work.tile([128, FB], F32, tag="f3", name="den")
            nc.vector.tensor_tensor(den[:], Sn32[:], Sp32[:], OP.subtract)
            nc.vector.tensor_scalar(den[:], den[:], EPS5, None, OP.add)
            rec = work.tile([128, FB], F32, tag="f4", name="rec")
            nc.vector.reciprocal(rec[:], den[:])
            num = work.tile([128, FB], F32, tag="f5", name="num")
            nc.vector.tensor_scalar(num[:], Sp32[:], -1.0, THR, OP.mult, OP.add)
            frac = work.tile([128, FB], F32, tag="f6", name="frac")
            nc.vector.tensor_tensor(frac[:], num[:], rec[:], OP.mult)
            # zero frac when idx == 0
            mi = work.tile([128, FB], F32, tag="f7", name="mi")
            nc.vector.tensor_scalar(mi[:], idx[:], 0.5, None, OP.is_ge)
            nc.vector.tensor_tensor(frac[:], frac[:], mi[:], OP.mult)
            idx0 = work.tile([128, FB], F32, tag="f8", name="idx0")
            nc.vector.tensor_scalar(idx0[:], idx[:], 1.0, 0.0, OP.subtract, OP.max)
            tval = work.tile([128, FB], F32, tag="f9", name="tval")
            nc.vector.tensor_tensor(tval[:], idx0[:], frac[:], OP.add)
            # seconds = found * (tval*0.01 - 1.2) + 1.2
            nc.vector.tensor_scalar(tval[:], tval[:], DT_MS / 1000.0, -1.2, OP.mult, OP.add)
            fnd = work.tile([128, FB], F32, tag="fa", name="fnd")
            nc.vector.tensor_scalar(fnd[:], Fo[:], 0.5, None, OP.is_ge)
            nc.vector.tensor_tensor(tval[:], tval[:], fnd[:], OP.mult)
            nc.vector.tensor_scalar(tval[:], tval[:], 1.2, None, OP.add)
            nc.sync.dma_start(out_d[:], tval[:])
    return nc


def _pin_act_table(nc):
    from concourse import hw_specs as _hs
    import concourse.bacc as _bacc
    full = dict(_hs.get_activation_tables(nc.m.arch))
    keep = "natural_log_exp_and_others"
    patched = {k: (v if k == keep else set()) for k, v in full.items()}
    _bacc.get_activation_tables = lambda arch: patched


last_results = None


def kernel(logits, input_scale, leak, self_excitation, inhibition, noise_std,
           proj_w, proj_b, noise_base):
    logits = np.asarray(logits, dtype=np.float32)
    noise_base = np.asarray(noise_base, dtype=np.float32)
    lk = _softplus(np.asarray(leak))
    se = _softplus(np.asarray(self_excitation))
    inh = float(_softplus(np.asarray(inhibition)))
    ns = float(_softplus(np.asarray(noise_std)))
    alpha = se + inh - lk  # [C]
    w00 = float(np.asarray(proj_w)[0, 0])
    pb0 = float(np.asarray(proj_b)[0])
    iscale = float(np.asarray(input_scale))

    # evidence + scaled noise, fp32 on host
    ev = (np.maximum(logits * iscale, 0.0) * w00 + pb0).astype(np.float32)  # [B,C]
    pn = noise_base * np.float32(ns) + ev[None, :, :]                       # [T,B,C]

    # A-tilde matrix on the 128-partition layout: partition p = c*G + g
    p_idx = np.arange(128)
    q_idx = np.arange(128)
    Am = (-inh / 5.0) * (p_idx[:, None] % G == q_idx[None, :] % G).astype(np.float32)
    Am[q_idx, q_idx] += alpha[q_idx // G] / 5.0
    wmats3 = np.zeros((17, 128, 128), dtype=np.float16)
    for j in range(15):
        wmats3[j] = (Am * (DEC ** (-(j + 1)))).astype(np.float16)
    wmats3[15] = Am.astype(np.float16)          # W_m1
    wmats3[16] = (Am * DEC).astype(np.float16)  # Wcar
    wmats = np.ascontiguousarray(wmats3.transpose(1, 0, 2)).reshape(128, 17 * 128)

    # dpn: epoch-scaled noise increments, fp16
    # dpn_t = 0.8^-u * pn_t - [u>0] * 0.8^-(u-1) * pn_{t-1},  u = t % K
    u_arr = np.arange(T) % K
    sc = (DEC ** (-u_arr.astype(np.float64))).astype(np.float32)          # [T]
    dpn = pn * sc[:, None, None]
    dpn[1:][u_arr[1:] > 0] -= pn[:-1][u_arr[1:] > 0] * sc[:-1][u_arr[1:] > 0, None, None]
    dpn = dpn.astype(np.float16)

    nc = bacc.Bacc("TRN2", target_bir_lowering=False, debug=False, num_devices=N_CORES)
    _build(nc)
    _pin_act_table(nc)
    nc.compile()

    in_maps = []
    for c in range(N_CORES):
        s = c * PB
        nz = dpn[:, s:s + PB, :].reshape(T, G, FB, C)
        nz = np.ascontiguousarray(nz.transpose(0, 3, 1, 2)).reshape(T, 128, FB)
        nz = np.ascontiguousarray(
            nz.reshape(T // CH, CH, 128, FB).transpose(0, 2, 1, 3)
        ).reshape(T // CH, 128, CH * FB)
        in_maps.append({"dpn": nz, "wmats": wmats})

    res = bass_utils.run_bass_kernel_spmd(nc, in_maps, core_ids=list(range(N_CORES)))
    global last_results
    last_results = res
    outs = []
    for c in range(N_CORES):
        o = res.results[c]["out"].reshape(C, G, FB)
        outs.append(o.transpose(1, 2, 0).reshape(PB, C))
    return np.concatenate(outs, axis=0)


# revision 17
# speedup vs baseline: 1.5813x; 1.0021x over previous
import os
import sys

for _p in ("/opt/trn_rl_repo", "/root/.axon_site/_ro/trn_rl_repo"):
    if os.path.isdir(_p) and _p not in sys.path:
        sys.path.insert(0, _p)

import numpy as np

import concourse.bass as bass
import concourse.mybir as mybir
from concourse.tile import TileContext
from concourse import bass_utils
from concourse import bacc

F16 = mybir.dt.float16
F32 = mybir.dt.float32
I16 = mybir.dt.int16
BF16 = mybir.dt.bfloat16
I32D = mybir.dt.int32
AF = mybir.ActivationFunctionType
OP = mybir.AluOpType

N_CORES = 8
BATCH = 65536
C = 4                  # classes
T = 120                # time steps
PB = BATCH // N_CORES  # batch per core = 8192
G = 32                 # groups (partitions = 4 classes x 32 groups)
FB = PB // G           # free-dim batch per partition = 256
K = 16                 # epoch length (0.8^-u stays fp16-safe)
NE = T // K            # epochs minus... T=120 -> 7 full epochs + 8 steps
OCT = 8                # octet length for bookkeeping
NOCT = T // OCT        # 15 octets
CH = 8                 # dpn steps per DMA chunk
DT_MS = 10.0
THR = 2.5              # threshold in S=5*acc units
EPS5 = 5e-9
FILL = 448             # filler matmul columns (PE p-state keep-alive)
NFILL = 1              # fillers per step
DEC = 0.8


def _softplus(x):
    return np.logaddexp(0.0, x.astype(np.float64)).astype(np.float32)


def _build(nc):
    dpn_d = nc.dram_tensor("dpn", [T // CH, 128, CH * FB], F16, kind="ExternalInput")
    w_d = nc.dram_tensor("wmats", [128, 17 * 128], F16, kind="ExternalInput")
    out_d = nc.dram_tensor("out", [128, FB], F32, kind="ExternalOutput")

    with TileContext(nc) as tc:
        with (
            tc.tile_pool(name="persist", bufs=1) as persist,
            tc.tile_pool(name="dpnp", bufs=3) as dpnp,
            tc.tile_pool(name="work", bufs=2) as work,
            tc.tile_pool(name="psum", bufs=1, space="PSUM") as psump,
        ):
            # --- stationary weights: W_j (j=0..14), W_m1=A, Wcar=0.8*A ---
            wall = persist.tile([128, 17 * 128], F16, name="wall")
            nc.sync.dma_start(wall[:], w_d[:])
            Wj = [wall[:, j * 128:(j + 1) * 128] for j in range(15)]
            Wm1 = wall[:, 15 * 128:16 * 128]
            Wcar = wall[:, 16 * 128:17 * 128]
            I32 = mybir.dt.int32
            ident = persist.tile([128, 128], F16, name="ident")
            rowi = persist.tile([128, 128], I32, name="rowi")
            nc.gpsimd.iota(rowi[:], [[1, 128]], base=0, channel_multiplier=0)
            coli = persist.tile([128, 1], I32, name="coli")
            nc.gpsimd.iota(coli[:], [[0, 1]], base=0, channel_multiplier=1)
            row = persist.tile([128, 128], F32, name="row")
            nc.vector.tensor_copy(row[:], rowi[:])
            col = persist.tile([128, 1], F32, name="col")
            nc.vector.tensor_copy(col[:], coli[:])
            nc.vector.tensor_scalar(ident[:], row[:], col[:], None, OP.is_equal)

            # --- state rings ---
            spr = persist.tile([128, 16 * FB], F16, name="spr")   # sp ring, 16 slots
            Sr = persist.tile([128, 16 * FB], F16, name="Sr")     # S ring, 16 slots
            nc.vector.memset(Sr[:], 0.0)
            Fo = persist.tile([128, FB], F16, name="Fo")          # crossed-by-octet flag
            nc.vector.memset(Fo[:], 0.0)
            cnt = persist.tile([128, FB], I32D, name="cnt")        # octets before crossing
            nc.vector.memset(cnt[:], 1)
            capS = persist.tile([128, FB], F16, name="capS")      # S before crossing octet
            capSP = persist.tile([128, OCT * FB], F16, name="capSP")  # sp's of crossing octet
            nc.vector.memset(capS[:], 0.0)
            nc.vector.memset(capSP[:], 0.0)
            nfo = [persist.tile([128, FB], I32D, name=f"nfo{i}") for i in range(2)]
            nfow = [persist.tile([128, OCT * FB], I16, name=f"nfow{i}") for i in range(2)]
            nc.vector.memset(nfo[0][:], 1)
            nc.vector.memset(nfow[0][:], 1)
            L1t = persist.tile([128, 4 * FB], F16, name="L1t")
            L2t = persist.tile([128, 2 * FB], F16, name="L2t")
            L3t = persist.tile([128, FB], F16, name="L3t")
            fseed = persist.tile([128, FILL], F16, name="fseed")
            nc.vector.memset(fseed[:], 0.125)

            # --- PSUM accumulators ---
            P0 = psump.tile([128, FB], F32, name="P0")
            P1 = psump.tile([128, FB], F32, name="P1")
            E0 = psump.tile([128, FB], F32, name="E0")
            E1 = psump.tile([128, FB], F32, name="E1")
            FP = psump.tile([128, FILL], F32, name="FP")
            Ps = [P0, P1]

            def fill_mm(n=1):
                for _ in range(n):
                    nc.tensor.matmul(FP[:], ident[:], fseed[:], start=True, stop=True)

            # warm up PE so it ramps to full speed before the scan
            for _ in range(10):
                fill_mm()

            dpn_t = [None] * (T // CH)
            dt0 = dpnp.tile([128, CH * FB], F16, tag="dpn", name="dp0")
            nc.sync.dma_start(dt0[:], dpn_d[0])
            dpn_t[0] = dt0
            nc.tensor.matmul(P0[:], ident[:], dt0[:, 0:FB], start=True, stop=False)
            spv = None  # sp tile AP of previous step (ring slot view)

            for t in range(T):
                u = t % K
                e = t // K
                P = Ps[e % 2]
                Pn = Ps[(e + 1) % 2]
                Etile = E0 if (t % 2 == 0) else E1

                # --- PE: carry for next epoch (start=True on Pn) must precede
                # the prefetched dpn matmul targeting Pn ---
                if u == K - 1 and t < T - 1:
                    sprev2 = Sr[:, ((t - 1) % 16) * FB:((t - 1) % 16 + 1) * FB]
                    nc.tensor.matmul(Pn[:], Wcar, sprev2, start=True, stop=False)

                # --- PE: z of previous sp into current P ---
                if t > 0 and u > 0:
                    nc.tensor.matmul(P[:], Wj[u - 1], spv, start=False, stop=False)

                # --- Act: softplus = Ln(1 + Exp(scale * P)) ---
                sslot = t % 16
                spc = spr[:, sslot * FB:(sslot + 1) * FB]
                nc.scalar.activation(Etile[:], P[:], AF.Exp, scale=float(DEC ** u))
                nc.scalar.activation(spc, Etile[:], AF.Ln, bias=1.0)
                spv = spc

                # --- PE: prefetch dpn matmul for step t+1 (runs during Ln_t,
                # emitted after Exp_t so the WAR dep points the right way) ---
                if t + 1 < T:
                    cin = (t + 1) // CH
                    if (t + 1) % CH == 0:
                        dtile = dpnp.tile([128, CH * FB], F16, tag="dpn", name=f"dp{cin}")
                        nc.sync.dma_start(dtile[:], dpn_d[cin])
                        dpn_t[cin] = dtile
                    dsl_n = dpn_t[cin][:, ((t + 1) % CH) * FB:((t + 1) % CH + 1) * FB]
                    un = (t + 1) % K
                    Ptgt = Pn if un == 0 else P
                    nc.tensor.matmul(Ptgt[:], ident[:], dsl_n, start=False, stop=False)

                # --- PE: last z of the epoch feeds the next epoch's P ---
                if u == K - 1 and t < T - 1:
                    nc.tensor.matmul(Pn[:], Wm1, spc, start=False, stop=False)

                # --- DVE: S update (writes S ring) ---
                Sprev = Sr[:, ((t - 1) % 16) * FB:((t - 1) % 16 + 1) * FB]
                Scur = Sr[:, (t % 16) * FB:((t % 16) + 1) * FB]
                if t == 0:
                    nc.vector.tensor_scalar(Scur, spc, 1.0, None, OP.mult)
                else:
                    nc.vector.scalar_tensor_tensor(Scur, Sprev, DEC, spc, OP.mult, OP.add)

                # --- deferred bookkeeping for the previous octet, one piece
                # per step so the DVE load stays smooth ---
                oc = t // OCT
                pos = t % OCT
                od = oc - 1
                if od >= 0:
                    odh = (od % 2) * OCT
                    if pos == 0:
                        spre = Sr[:, ((8 * od - 1) % 16) * FB:(((8 * od - 1) % 16) + 1) * FB]
                        nc.vector.copy_predicated(capS[:], nfo[od % 2][:], spre)
                    elif pos == 1:
                        SrH = Sr[:, odh * FB:(odh + OCT) * FB]
                        nc.vector.tensor_tensor(
                            L1t[:], SrH[:, 0:4 * FB], SrH[:, 4 * FB:8 * FB], OP.max)
                    elif pos == 2:
                        nc.vector.tensor_tensor(
                            L2t[:], L1t[:, 0:2 * FB], L1t[:, 2 * FB:4 * FB], OP.max)
                    elif pos == 3:
                        nc.vector.tensor_tensor(
                            L3t[:], L2t[:, 0:FB], L2t[:, FB:2 * FB], OP.max)
                    elif pos == 4:
                        nc.vector.scalar_tensor_tensor(
                            Fo[:], L3t[:], THR, Fo[:], OP.is_ge, OP.max)
                    elif pos == 5:
                        nc.vector.tensor_scalar(nfo[oc % 2][:], Fo[:], 0.5, None, OP.is_lt)
                        nc.gpsimd.tensor_tensor(cnt[:], cnt[:], nfo[oc % 2][:], OP.add)
                    elif pos == 6:
                        nc.vector.tensor_copy(
                            nfow[oc % 2][:].rearrange("p (a b) -> p a b", a=OCT),
                            nfo[oc % 2][:].unsqueeze(1).broadcast_to([128, OCT, FB]),
                        )
                    elif pos == 7:
                        nc.vector.copy_predicated(
                            capSP[:], nfow[od % 2][:], spr[:, odh * FB:(odh + OCT) * FB])

                # --- PE filler to keep the p-state hot ---
                fill_mm(NFILL)

            # tail: deferred bookkeeping for the last octet (od = 14)
            od = NOCT - 1
            odh = (od % 2) * OCT
            spre = Sr[:, ((8 * od - 1) % 16) * FB:(((8 * od - 1) % 16) + 1) * FB]
            nc.vector.copy_predicated(capS[:], nfo[od % 2][:], spre)
            SrH = Sr[:, odh * FB:(odh + OCT) * FB]
            nc.vector.tensor_tensor(L1t[:], SrH[:, 0:4 * FB], SrH[:, 4 * FB:8 * FB], OP.max)
            nc.vector.tensor_tensor(L2t[:], L1t[:, 0:2 * FB], L1t[:, 2 * FB:4 * FB], OP.max)
            nc.vector.tensor_tensor(L3t[:], L2t[:, 0:FB], L2t[:, FB:2 * FB], OP.max)
            nc.vector.scalar_tensor_tensor(Fo[:], L3t[:], THR, Fo[:], OP.is_ge, OP.max)
            nc.vector.copy_predicated(
                capSP[:], nfow[od % 2][:], spr[:, odh * FB:(odh + OCT) * FB])

            # ================= epilogue =================
            # reconstruct R_j (S values inside the crossing octet)
            Rj = []
            Rprev = capS
            for j in range(OCT):
                R = persist.tile([128, FB], F16, name=f"R{j}")
                nc.vector.scalar_tensor_tensor(
                    R[:], Rprev[:] if j == 0 else Rprev, DEC,
                    capSP[:, j * FB:(j + 1) * FB], OP.mult, OP.add)
                Rj.append(R)
                Rprev = R[:]

            # j* = count of leading below-threshold cummax over R_j
            jstar = work.tile([128, FB], F32, tag="ep2", name="jstar")
            nc.vector.memset(jstar[:], 0.0)
            cm = work.tile([128, FB], F16, tag="ep5", name="cm")
            for j in range(OCT):
                if j == 0:
                    nc.vector.tensor_copy(cm[:], Rj[0][:])
                else:
                    nc.vector.tensor_tensor(cm[:], cm[:], Rj[j][:], OP.max)
                nc.vector.scalar_tensor_tensor(
                    jstar[:], cm[:], THR, jstar[:], OP.is_lt, OP.add)
            cntf = work.tile([128, FB], F32, tag="ep4", name="cntf")
            nc.vector.tensor_copy(cntf[:], cnt[:])
            # idx = 8*(cnt-1) + j*
            idx = work.tile([128, FB], F32, tag="ep", name="idx")
            nc.vector.tensor_scalar(idx[:], cntf[:], 8.0, -8.0, OP.mult, OP.add)
            nc.vector.tensor_tensor(idx[:], idx[:], jstar[:], OP.add)

            Sn = persist.tile([128, FB], F16, name="Sn")
            Sp = persist.tile([128, FB], F16, name="Sp")
            nc.vector.memset(Sn[:], 1.0)
            nc.vector.memset(Sp[:], 0.0)
            for j in range(OCT):
                pj = work.tile([128, FB], I16, tag="pj", name=f"pj{j}")
                nc.vector.tensor_scalar(pj[:], jstar[:], float(j), None, OP.is_equal)
                nc.vector.copy_predicated(Sn[:], pj[:], Rj[j][:])
                nc.vector.copy_predicated(Sp[:], pj[:], capS[:] if j == 0 else Rj[j - 1][:])

            # frac and final time (fp32)
            Sp32 = work.tile([128, FB], F32, tag="f1", name="Sp32")
            nc.vector.tensor_copy(Sp32[:], Sp[:])
            Sn32 = work.tile([128, FB], F32, tag="f2", name="Sn32")
            nc.vector.tensor_copy(Sn32[:], Sn[:])
            den = work.tile([128, FB], F32, tag="f3", name="den")
            nc.vector.tensor_tensor(den[:], Sn32[:], Sp32[:], OP.subtract)
            nc.vector.tensor_scalar(den[:], den[:], EPS5, None, OP.add)
            rec = work.tile([128, FB], F32, tag="f4", name="rec")
            nc.vector.reciprocal(rec[:], den[:])
            num = work.tile([128, FB], F32, tag="f5", name="num")
            nc.vector.tensor_scalar(num[:], Sp32[:], -1.0, THR, OP.mult, OP.add)
            frac = work.tile([128, FB], F32, tag="f6", name="frac")
            nc.vector.tensor_tensor(frac[:], num[:], rec[:], OP.mult)
            # zero frac when idx == 0
            mi = work.tile([128, FB], F32, tag="f7", name="mi")
            nc.vector.tensor_scalar(mi[:], idx[:], 0.5, None, OP.is_ge)
            nc.vector.tensor_tensor(frac[:], frac[:], mi[:], OP.mult)
            idx0 = work.tile([128, FB], F32, tag="f8", name="idx0")
            nc.vector.tensor_scalar(idx0[:], idx[:], 1.0, 0.0, OP.subtract, OP.max)
            tval = work.tile([128, FB], F32, tag="f9", name="tval")
            nc.vector.tensor_tensor(tval[:], idx0[:], frac[:], OP.add)
            # seconds = found * (tval*0.01 - 1.2) + 1.2
            nc.vector.tensor_scalar(tval[:], tval[:], DT_MS / 1000.0, -1.2, OP.mult, OP.add)
            fnd = work.tile([128, FB], F32, tag="fa", name="fnd")
            nc.vector.tensor_scalar(fnd[:], Fo[:], 0.5, None, OP.is_ge)
            nc.vector.tensor_tensor(tval[:], tval[:], fnd[:], OP.mult)
            nc.vector.tensor_scalar(tval[:], tval[:], 1.2, None, OP.add)
            nc.sync.dma_start(out_d[:], tval[:])
    return nc


def _pin_act_table(nc):
    from concourse import hw_specs as _hs
    import concourse.bacc as _bacc
    full = dict(_hs.get_activation_tables(nc.m.arch))
    keep = "natural_log_exp_and_others"
    patched = {k: (v if k == keep else set()) for k, v in full.items()}
    _bacc.get_activation_tables = lambda arch: patched


last_results = None


def kernel(logits, input_scale, leak, self_excitation, inhibition, noise_std,
           proj_w, proj_b, noise_base):
    logits = np.asarray(logits, dtype=np.float32)
    noise_base = np.asarray(noise_base, dtype=np.float32)
    lk = _softplus(np.asarray(leak))
    se = _softplus(np.asarray(self_excitation))
    inh = float(_softplus(np.asarray(inhibition)))
    ns = float(_softplus(np.asarray(noise_std)))
    alpha = se + inh - lk  # [C]
    w00 = float(np.asarray(proj_w)[0, 0])
    pb0 = float(np.asarray(proj_b)[0])
    iscale = float(np.asarray(input_scale))

    # evidence + scaled noise, fp32 on host
    ev = (np.maximum(logits * iscale, 0.0) * w00 + pb0).astype(np.float32)  # [B,C]
    pn = noise_base * np.float32(ns) + ev[None, :, :]                       # [T,B,C]

    # A-tilde matrix on the 128-partition layout: partition p = c*G + g
    p_idx = np.arange(128)
    q_idx = np.arange(128)
    Am = (-inh / 5.0) * (p_idx[:, None] % G == q_idx[None, :] % G).astype(np.float32)
    Am[q_idx, q_idx] += alpha[q_idx // G] / 5.0
    wmats3 = np.zeros((17, 128, 128), dtype=np.float16)
    for j in range(15):
        wmats3[j] = (Am * (DEC ** (-(j + 1)))).astype(np.float16)
    wmats3[15] = Am.astype(np.float16)          # W_m1
    wmats3[16] = (Am * DEC).astype(np.float16)  # Wcar
    wmats = np.ascontiguousarray(wmats3.transpose(1, 0, 2)).reshape(128, 17 * 128)

    # dpn: epoch-scaled noise increments, fp16
    # dpn_t = 0.8^-u * pn_t - [u>0] * 0.8^-(u-1) * pn_{t-1},  u = t % K
    u_arr = np.arange(T) % K
    sc = (DEC ** (-u_arr.astype(np.float64))).astype(np.float32)          # [T]
    dpn = pn * sc[:, None, None]
    dpn[1:][u_arr[1:] > 0] -= pn[:-1][u_arr[1:] > 0] * sc[:-1][u_arr[1:] > 0, None, None]
    dpn = dpn.astype(np.float16)

    nc = bacc.Bacc("TRN2", target_bir_lowering=False, debug=False, num_devices=N_CORES)
    _build(nc)
    _pin_act_table(nc)
    nc.compile()

    in_maps = []
    for c in range(N_CORES):
        s = c * PB
        nz = dpn[:, s:s + PB, :].reshape(T, G, FB, C)
        nz = np.ascontiguousarray(nz.transpose(0, 3, 1, 2)).reshape(T, 128, FB)
        nz = np.ascontiguousarray(
            nz.reshape(T // CH, CH, 128, FB).transpose(0, 2, 1, 3)
        ).reshape(T // CH, 128, CH * FB)
        in_maps.append({"dpn": nz, "wmats": wmats})

    res = bass_utils.run_bass_kernel_spmd(nc, in_maps, core_ids=list(range(N_CORES)))
    global last_results
    last_results = res
    outs = []
    for c in range(N_CORES):
        o = res.results[c]["out"].reshape(C, G, FB)
        outs.append(o.transpose(1, 2, 0).reshape(PB, C))
    return np.concatenate(outs, axis=0)
